# revision 1
# baseline (speedup 1.0000x reference)
"""InterpretableMultiHeadAttention on 8 Trainium2 NeuronCores (Bass/Tile).

Sharding: core c -> batch b = c//2, head-group hg = c%2 (8 of 16 heads).
Math folding (exact up to fp rounding):
  v' = v @ Wv.T + bv, x = sum_h attn_h @ v'_h, out = x @ Wo.T + bo
  Since softmax rows sum to 1:  attn @ (1 bv^T) = 1 bv^T, so
  out = (sum_h attn_h @ v_h) @ (Wo @ Wv).T + (H * Wo @ bv + bo)
The 1/sqrt(d) score scale folds into Wq/bq.

Device schedule: query-block (iq) outer loop; per block: scores^T with
2-head row-packing (d=64 contraction), exp on ScalarE, PV matmul with a
per-head ones-column at 64+h so softmax denominators land on PSUM
partitions 64..71 (rows elsewhere are exact zeros, so an 8-row add
accumulates them without clobbering). Each block's softmax division +
head-sum + Wov projection overlaps the next block's attention.
Host sums the two partial projections per batch and adds the bias.
"""
import numpy as np

N_OUT = 1024
N_HEADS = 16
D_K = 64
B = 4
S = 2048
FC = 8          # 1024 contraction f-chunks of 128 (projections)
PAIRS = 4       # 8 local heads as 4 row-packed pairs
NMM = 512       # matmul moving free dim
JC = S // 128   # key chunks of 128
IQ = S // NMM   # query blocks of 512
MV = 72         # PV lhsT width: 64 v dims + ones column at 64+h (h<8)

_CACHE = {}


def _build_nc():
    from contextlib import ExitStack
    import concourse.bass as bass
    import concourse.bacc as bacc
    import concourse.tile as tile
    import concourse.mybir as mybir
    from concourse.dve_ops import RECIPROCAL_APPROX_FAST, RECIP_APPROX_FAST_CONSTS

    f16 = mybir.dt.float16
    f32 = mybir.dt.float32

    nc = bacc.Bacc("TRN2", target_bir_lowering=False, debug=False, num_devices=8)

    xq_d = nc.dram_tensor("xq", [FC, 128, S], f16, kind="ExternalInput")
    xk_d = nc.dram_tensor("xk", [FC, 128, S], f16, kind="ExternalInput")
    wq_d = nc.dram_tensor("wq", [128, FC, 512], f16, kind="ExternalInput")
    wk_d = nc.dram_tensor("wk", [128, FC, 512], f16, kind="ExternalInput")
    bq_d = nc.dram_tensor("bq", [128, PAIRS], f32, kind="ExternalInput")
    bk_d = nc.dram_tensor("bk", [128, PAIRS], f32, kind="ExternalInput")
    vv_d = nc.dram_tensor("vv", [PAIRS, 128, JC, 2, MV], f16, kind="ExternalInput")
    wov_d = nc.dram_tensor("wov", [64, N_OUT], f16, kind="ExternalInput")
    out_d = nc.dram_tensor("outT", [8, 128, S], f32, kind="ExternalOutput")
    den_d = nc.dram_tensor("den_scratch", [IQ, 8, NMM], f32)  # bounce

    with tile.TileContext(nc) as tc, ExitStack() as ctx:
        const = ctx.enter_context(tc.tile_pool(name="const", bufs=1))
        qkall = ctx.enter_context(tc.tile_pool(name="qkall", bufs=1))
        epool = ctx.enter_context(tc.tile_pool(name="epool", bufs=3))
        blk = ctx.enter_context(tc.tile_pool(name="blk", bufs=2))
        fin = ctx.enter_context(tc.tile_pool(name="fin", bufs=2))
        tree = ctx.enter_context(tc.tile_pool(name="tree", bufs=1))
        ost_p = ctx.enter_context(tc.tile_pool(name="ost_p", bufs=2))
        ps_mm = ctx.enter_context(tc.tile_pool(name="ps_mm", bufs=3, space="PSUM"))
        ps_y = ctx.enter_context(tc.tile_pool(name="ps_y", bufs=2, space="PSUM"))
        xctx = ExitStack()
        xstage = xctx.enter_context(tc.tile_pool(name="xstage", bufs=1))

        # ---- input loads (proj-critical first) ----
        wq_sb = const.tile([128, FC, 512], f16, tag="wq")
        nc.sync.dma_start(out=wq_sb[:], in_=wq_d[:])
        bq_sb = const.tile([128, PAIRS], f32, tag="bq")
        nc.sync.dma_start(out=bq_sb[:], in_=bq_d[:])
        bk_sb = const.tile([128, PAIRS], f32, tag="bk")
        nc.gpsimd.dma_start(out=bk_sb[:], in_=bk_d[:])
        wk_sb = const.tile([128, FC, 512], f16, tag="wk")
        engs = [nc.sync, nc.scalar, nc.gpsimd]
        xq_sb, xk_sb = [], []
        ei = 1
        for f in range(FC):
            t = xstage.tile([128, S], f16, tag=f"xq{f}")
            engs[ei % 3].dma_start(out=t[:], in_=xq_d[f])
            ei += 1
            xq_sb.append(t)
        nc.scalar.dma_start(out=wk_sb[:], in_=wk_d[:])
        for f in range(FC):
            t = xstage.tile([128, S], f16, tag=f"xk{f}")
            engs[ei % 3].dma_start(out=t[:], in_=xk_d[f])
            ei += 1
            xk_sb.append(t)
        vv_sb = []
        for p in range(PAIRS):
            t = qkall.tile([128, JC, 2, MV], f16, tag=f"vv{p}")
            nc.sync.dma_start(out=t[:], in_=vv_d[p])
            vv_sb.append(t)
        wov_sb = const.tile([64, N_OUT], f16, tag="wov")
        nc.sync.dma_start(out=wov_sb[:], in_=wov_d[:])

        qT, kT = {}, {}

        def proj(p):
            qT[p] = qkall.tile([128, S], f16, tag=f"qT{p}", name=f"qT{p}")
            kT[p] = qkall.tile([128, S], f16, tag=f"kT{p}", name=f"kT{p}")
            for dst, w_sb, b_sb, x_sb in (
                (qT[p], wq_sb, bq_sb, xq_sb),
                (kT[p], wk_sb, bk_sb, xk_sb),
            ):
                for sc in range(S // 1024):
                    ps = ps_mm.tile([128, 1024], f32, tag="mm")
                    for hf in range(2):
                        c0 = sc * 1024 + hf * 512
                        for f in range(FC):
                            nc.tensor.matmul(
                                out=ps[:, hf * 512:(hf + 1) * 512],
                                lhsT=w_sb[:, f, p * 128:(p + 1) * 128],
                                rhs=x_sb[f][:, c0:c0 + 512],
                                start=(f == 0),
                                stop=(f == FC - 1),
                            )
                    nc.vector.tensor_scalar_add(
                        out=dst[:, sc * 1024:(sc + 1) * 1024],
                        in0=ps[:],
                        scalar1=b_sb[:, p:p + 1],
                    )

        def outproj(iq, y16):
            i0 = iq * NMM
            for m in range(8):
                po = ps_mm.tile([128, 1024], f32, tag="mm")
                nc.tensor.matmul(
                    out=po[:, :NMM],
                    lhsT=wov_sb[:, m * 128:(m + 1) * 128],
                    rhs=y16[:],
                    start=True, stop=True,
                )
                ost = ost_p.tile([128, NMM], f32, tag="ost")
                nc.vector.tensor_copy(out=ost[:], in_=po[:, :NMM])
                nc.sync.dma_start(out=out_d[m][:, i0:i0 + NMM], in_=ost[:])

        def bcast_recips(iq, den_t):
            """Bounce reciprocals through DRAM and broadcast each head's row
            across 64 partitions (same gpsimd queue -> FIFO-ordered)."""
            nc.gpsimd.dma_start(out=den_d[iq], in_=den_t[64:72, :])
            rbs = []
            for h in range(8):
                rb = fin.tile([64, NMM], f32, tag=f"rb{h}", name=f"rb{h}")
                row = den_d[iq, h:h + 1, :]
                bc = bass.AP(tensor=row.tensor, offset=row.offset,
                             ap=[[0, 64]] + row.ap[1:])
                nc.gpsimd.dma_start(out=rb[:], in_=bc)
                rbs.append(rb)
            return rbs

        def finalize_div(y_blk, rbs):
            """Divide each head by its denominator and tree-sum into y16."""
            accs = []
            for h in range(8):
                if h < 4:
                    a = tree.tile([64, NMM], f32, tag=f"acc{h}", name=f"acc{h}")
                    nc.vector.tensor_mul(out=a[:], in0=y_blk[:, h, :], in1=rbs[h][:])
                    accs.append(a)
                else:
                    t = tree.tile([64, NMM], f32, tag="tmp")
                    nc.vector.tensor_mul(out=t[:], in0=y_blk[:, h, :], in1=rbs[h][:])
                    nc.vector.tensor_add(
                        out=accs[h - 4][:], in0=accs[h - 4][:], in1=t[:])
            nc.vector.tensor_add(out=accs[0][:], in0=accs[0][:], in1=accs[1][:])
            nc.vector.tensor_add(out=accs[2][:], in0=accs[2][:], in1=accs[3][:])
            y16 = blk.tile([64, NMM], f16, tag="y16")
            nc.vector.tensor_add(out=y16[:], in0=accs[0][:], in1=accs[2][:])
            return y16

        pend_fin = None     # (iq, y_blk, rbs) awaiting divide+head-sum
        pend_out = None     # (iq, y16) awaiting output projection
        for iq in range(IQ):
            i0 = iq * NMM
            y_blk = blk.tile([64, 8, NMM], f16, tag="yblk")
            den_t = blk.tile([128, NMM], f32, tag="den")
            nc.vector.memset(den_t[64:72, :], 0.0)
            for p in range(PAIRS):
                if iq == 0:
                    proj(p)
                if p == 0 and pend_fin is not None:
                    # deferred emission: the DVE reaches these muls after
                    # this block's first copies (keeps PSUM slots cycling)
                    pend_out = (pend_fin[0],
                                finalize_div(pend_fin[1], pend_fin[2]))
                    pend_fin = None
                if p == 2 and pend_out is not None:
                    # deferred emission: PE reaches these matmuls only after
                    # their inputs are long ready (no in-order queue stall)
                    outproj(*pend_out)
                    pend_out = None
                hA, hB = 2 * p, 2 * p + 1
                yA = ps_y.tile([MV, NMM], f32, tag="yab")
                yB = ps_y.tile([MV, NMM], f32, tag="yab")
                for jc in range(JC):
                    j0 = jc * 128
                    sAB = ps_mm.tile([128, 1024], f32, tag="mm")
                    nc.tensor.matmul(
                        out=sAB[:, :NMM],
                        lhsT=kT[p][0:64, j0:j0 + 128],
                        rhs=qT[p][0:64, i0:i0 + NMM],
                        start=True, stop=True,
                        tile_position=(0, 0),
                    )
                    nc.tensor.matmul(
                        out=sAB[:, NMM:],
                        lhsT=kT[p][64:128, j0:j0 + 128],
                        rhs=qT[p][64:128, i0:i0 + NMM],
                        start=True, stop=True,
                        tile_position=(64, 0),
                    )
                    eAB = epool.tile([128, 1024], f16, tag="e")
                    nc.scalar.activation(
                        out=eAB[:], in_=sAB[:],
                        func=mybir.ActivationFunctionType.Exp,
                    )
                    nc.tensor.matmul(
                        out=yA[:],
                        lhsT=vv_sb[p][:, jc, 0, :],
                        rhs=eAB[:, :NMM],
                        start=(jc == 0), stop=(jc == JC - 1),
                        skip_group_check=True,
                    )
                    nc.tensor.matmul(
                        out=yB[:],
                        lhsT=vv_sb[p][:, jc, 1, :],
                        rhs=eAB[:, NMM:],
                        start=(jc == 0), stop=(jc == JC - 1),
                        skip_group_check=True,
                    )
                # numerators (fp16); rows 64..71 of y are zero except row
                # 64+h, so the 8-row add accumulates den without clobbering
                for y, h in ((yA, hA), (yB, hB)):
                    nc.vector.tensor_copy(out=y_blk[:, h, :], in_=y[0:64, :])
                    nc.vector.tensor_add(
                        out=den_t[64:72, :],
                        in0=den_t[64:72, :],
                        in1=y[64:72, :])
            if iq == 0:
                xctx.close()  # release x staging after last projection

            nc.vector.reciprocal(out=den_t[64:72, :], in_=den_t[64:72, :])
            pend_fin = (iq, y_blk, bcast_recips(iq, den_t))
        # drain the last block
        outproj(pend_fin[0], finalize_div(pend_fin[1], pend_fin[2]))

    nc.compile()
    return nc


def _prep(queries, keys, values, Wq, bq, Wk, bk, Wv, bv, Wo, bo):
    """Host-side sharding/layout prep. Returns (in_maps, bo_p)."""
    queries = np.asarray(queries, np.float32)
    keys = np.asarray(keys, np.float32)
    values = np.asarray(values, np.float32)
    Wq = np.asarray(Wq, np.float32)
    bq = np.asarray(bq, np.float32)
    Wk = np.asarray(Wk, np.float32)
    bk = np.asarray(bk, np.float32)
    Wv = np.asarray(Wv, np.float32)
    bv = np.asarray(bv, np.float32)
    Wo = np.asarray(Wo, np.float32)
    bo = np.asarray(bo, np.float32)

    scale = 1.0 / np.sqrt(np.float32(D_K))
    Wq_s = Wq * scale
    bq_s = bq * scale
    Wov = Wo @ Wv                       # [1024, 64]
    bo_p = bo + N_HEADS * (Wo @ bv)     # [1024]
    wov_h = np.ascontiguousarray(Wov.T.astype(np.float16))  # [64, 1024]

    in_maps = []
    for c in range(8):
        b = c // 2
        hg = c % 2
        hsl = slice(hg * 512, (hg + 1) * 512)
        xq = np.ascontiguousarray(
            queries[b].T.astype(np.float16).reshape(FC, 128, S))
        xk = np.ascontiguousarray(
            keys[b].T.astype(np.float16).reshape(FC, 128, S))
        wq = np.ascontiguousarray(
            Wq_s[hsl].T.astype(np.float16).reshape(FC, 128, 512).transpose(1, 0, 2))
        wk = np.ascontiguousarray(
            Wk[hsl].T.astype(np.float16).reshape(FC, 128, 512).transpose(1, 0, 2))
        bq_c = np.ascontiguousarray(bq_s[hsl].reshape(PAIRS, 128).T)
        bk_c = np.ascontiguousarray(bk[hsl].reshape(PAIRS, 128).T)
        vv = np.zeros((128, JC, 8, MV), np.float16)
        vv[:, :, :, :64] = (
            values[b][:, hsl].astype(np.float16)
            .reshape(JC, 128, 8, 64).transpose(1, 0, 2, 3))
        for h in range(8):
            vv[:, :, h, 64 + h] = 1.0
        # [PAIRS, 128, JC, 2, MV] so each pair's slice is one contiguous DMA
        vv = vv.reshape(128, JC, PAIRS, 2, MV).transpose(2, 0, 1, 3, 4)
        in_maps.append({
            "xq": xq, "xk": xk, "wq": wq, "wk": wk,
            "bq": bq_c, "bk": bk_c, "vv": np.ascontiguousarray(vv),
            "wov": wov_h,
        })
    return in_maps, bo_p


def _build_in_maps(inputs):
    return _prep(**inputs)[0]


def _gather(results, bo_p):
    out = np.empty((B, S, N_OUT), np.float32)
    for b in range(B):
        oT = results[2 * b]["outT"] + results[2 * b + 1]["outT"]
        out[b] = oT.reshape(N_OUT, S).T + bo_p
    return out


def kernel(queries, keys, values, Wq, bq, Wk, bk, Wv, bv, Wo, bo):
    from concourse.bass_utils import run_bass_kernel_spmd

    in_maps, bo_p = _prep(queries, keys, values, Wq, bq, Wk, bk, Wv, bv, Wo, bo)
    if "nc" not in _CACHE:
        _CACHE["nc"] = _build_nc()
    res = run_bass_kernel_spmd(_CACHE["nc"], in_maps, core_ids=list(range(8)))
    return _gather(res.results, bo_p)

